# revision 1
# baseline (speedup 1.0000x reference)
"""SchNet encoder (CASchNetEncoder) distributed Bass kernel for 8 Trainium2 cores.

Strategy:
  - Nodes sharded into 8 contiguous blocks of 5000; each core owns the edges
    whose dst lands in its block (graph partition by destination).
  - Edges with length > cutoff are dropped on the host (exact: their gate C=0).
  - Per core, edges are bucketed by 128-node destination windows and padded to
    a tile schedule that is identical across cores (SPMD: one program).
  - Per layer: x = h @ lin1.T computed on owned nodes chunk-by-chunk inside
    the node path (per-chunk DMA overlaps compute, the AllGather fires as
    early as possible); x[src] fetched with batched dma_gather calls at
    node-PAIR granularity (512-byte rows; <=1024 indices per call since the
    SWDGE descriptor ring holds 128 entries/engine); the correct half of
    each pair is selected with a host-precomputed parity scalar
    (scalar-engine copy with scale + one fused DVE select).
  - Scatter-add done as one-hot matmuls accumulating in PSUM per destination
    window; the one-hot tiles are host-precomputed and streamed from DRAM.
    The filter term and the bias term share one [128, 256] accumulation.
  - ShiftedSoftplus log(2) shifts and biases are folded into downstream
    weights/biases on the host.
"""

import numpy as np
import ml_dtypes

from concourse import bass, mybir
import concourse.bacc as bacc
import concourse.tile as tile
from concourse.bass_utils import run_bass_kernel_spmd
from concourse.masks import make_identity

# The activation-table pass picks the first table set containing each
# function: Exp -> exp_and_others, Ln -> natural_log, so a kernel that
# alternates Exp/Ln reloads the LUT on nearly every instruction (~1.3 us
# each).  Steer Exp/Ln/Copy to the one set that holds all three
# (natural_log_exp_and_others) so the table is loaded exactly once.
_COMBINED_SET = "natural_log_exp_and_others"
_STEERED = {
    mybir.ActivationFunctionType.Exp,
    mybir.ActivationFunctionType.Ln,
    mybir.ActivationFunctionType.Copy,
}
if not getattr(bacc, "_act_tables_steered", False):
    _orig_get_tables = bacc.get_activation_tables

    def _steered_get_tables(arch):
        tables = dict(_orig_get_tables(arch))
        return {
            name: (fns if name == _COMBINED_SET else fns - _STEERED)
            for name, fns in tables.items()
        }

    bacc.get_activation_tables = _steered_get_tables
    bacc._act_tables_steered = True

BF16 = mybir.dt.bfloat16
F32 = mybir.dt.float32
I16 = mybir.dt.int16

NP_BF16 = ml_dtypes.bfloat16

NCORES = 8
N = 40000
E = 640000
H = 128
FLT = 128
EC = 100
L = 6
IN = 5
CUTOFF = 10.0
LOG2 = float(np.log(2.0))

P = 128
NLOC = N // NCORES            # 5000 nodes per core
NW = (NLOC + P - 1) // P      # 40 destination windows per core
NPAD = NW * P                 # 5120 padded node columns per core
NPAIR = NCORES * NPAD // 2    # 20480 node pairs in the gather table
GMAX = 8                      # max tiles (1024 indices) per dma_gather call


def _host_prep(inputs):
    """Partition/sort/pad edges, fold biases, build per-core device arrays."""
    z = np.asarray(inputs["z"], np.float32)
    edge_index = np.asarray(inputs["edge_index"]).astype(np.int64)
    edge_length = np.asarray(inputs["edge_length"], np.float32)
    edge_attr = np.asarray(inputs["edge_attr"], np.float32)

    live = edge_length <= CUTOFF
    src = edge_index[0][live]
    dst = edge_index[1][live]
    attr = edge_attr[live]

    owner = dst // NLOC
    ldst = dst - owner * NLOC
    win = ldst // P
    # gather-table row index for each source node (block-padded layout)
    srow = (src // NLOC) * NPAD + (src % NLOC)

    # per (core, window) counts -> shared (max over cores) tile schedule
    cnt = np.zeros((NCORES, NW), np.int64)
    np.add.at(cnt, (owner, win), 1)
    tiles_per_win = np.maximum(1, -(-cnt.max(axis=0) // P))  # ceil, >=1
    tstart = np.zeros(NW + 1, np.int64)
    tstart[1:] = np.cumsum(tiles_per_win)
    ttot = int(tstart[-1])
    ep = ttot * P

    # order by (owner, window, srow): ascending gather addresses per window
    order = np.lexsort((srow, win, owner))
    so, sw = owner[order], win[order]
    sattr = attr[order]
    sldst = ldst[order]
    ssrow = srow[order]
    gkey = so * NW + sw
    gstart_all = np.zeros(NCORES * NW, np.int64)
    np.cumsum(np.bincount(gkey, minlength=NCORES * NW)[:-1], out=gstart_all[1:])
    rank = np.arange(len(so)) - gstart_all[gkey]
    tile_idx = tstart[sw] + rank // P
    part_idx = rank % P
    slot = tile_idx * P + part_idx

    attrT = np.zeros((NCORES, EC, ep), NP_BF16)
    attrT[so, :, slot] = sattr.astype(NP_BF16)

    # one-hot scatter tiles: oh[p, t*P + d] = (dst offset of slot (p,t) == d)
    ohall = np.zeros((NCORES, P, ep), NP_BF16)
    ohall[so, part_idx, tile_idx * P + (sldst - sw * P)] = NP_BF16(1)

    # pair-granularity gather indices (int16) and parity selectors
    idx16 = np.zeros((NCORES, ep), np.int16)
    idx16[so, slot] = (ssrow // 2).astype(np.int16)
    gidx = np.ascontiguousarray(
        np.tile(idx16.reshape(NCORES, ttot * 8, 16).transpose(0, 2, 1),
                (1, 8, 1))
    )  # [NCORES, 128, ttot*8]: call-index i at [i%16 (+16k), i//16]
    par1 = np.zeros((NCORES, P, ttot), np.float32)
    par1[so, part_idx, tile_idx] = (ssrow % 2).astype(np.float32)
    par0 = np.zeros((NCORES, P, ttot), np.float32)
    par0[so, part_idx, tile_idx] = 1.0 - (ssrow % 2)

    # weights with folded shifts
    nW1 = np.asarray(inputs["nn_W1"], np.float32)
    nb1 = np.asarray(inputs["nn_b1"], np.float32)
    nW2 = np.asarray(inputs["nn_W2"], np.float32)
    nb2 = np.asarray(inputs["nn_b2"], np.float32)
    l1W = np.asarray(inputs["lin1_W"], np.float32)
    l2W = np.asarray(inputs["lin2_W"], np.float32)
    l2b = np.asarray(inputs["lin2_b"], np.float32)
    lW = np.asarray(inputs["lin_W"], np.float32)
    lb = np.asarray(inputs["lin_b"], np.float32)
    emblin_W = np.asarray(inputs["emblin_W"], np.float32)
    emblin_b = np.asarray(inputs["emblin_b"], np.float32)

    wx = {
        "nW1T": np.ascontiguousarray(nW1.transpose(0, 2, 1)).astype(NP_BF16),
        "nb1": np.ascontiguousarray(nb1.T),  # [FLT, L] f32
        "nW2T": np.ascontiguousarray(nW2.transpose(0, 2, 1)).astype(NP_BF16),
        # [L, P, FLT]: nb2' = nb2 - log2 * rowsum(nW2), broadcast down partitions
        "nb2row": np.ascontiguousarray(
            np.broadcast_to(
                (nb2 - LOG2 * nW2.sum(axis=2))[:, None, :], (L, P, FLT)
            )
        ).astype(NP_BF16),
        "l1WT": np.ascontiguousarray(l1W.transpose(0, 2, 1)).astype(NP_BF16),
        "l2WT": np.ascontiguousarray(l2W.transpose(0, 2, 1)).astype(NP_BF16),
        "l2b": np.ascontiguousarray(l2b.T),  # [H, L] f32
        "lWT": np.ascontiguousarray(lW.transpose(0, 2, 1)).astype(NP_BF16),
        "lbp": np.ascontiguousarray((lb - LOG2 * lW.sum(axis=2)).T),  # [H, L]
        "emblinT": np.ascontiguousarray(emblin_W.T),  # [IN, H] f32
    }

    featsT = np.zeros((NCORES, IN, NPAD), np.float32)
    ptembT = np.zeros((NCORES, H, NPAD), np.float32)
    for c in range(NCORES):
        blk = z[c * NLOC : (c + 1) * NLOC]
        featsT[c, :, :NLOC] = blk[:, :IN].T
        ptembT[c, :, :NLOC] = blk[:, IN:].T + emblin_b[:, None]

    sched = dict(tstart=tstart, ttot=ttot, ep=ep)
    percore = dict(attrT=attrT, ohall=ohall, gidx=gidx, par0=par0, par1=par1,
                   featsT=featsT, ptembT=ptembT)
    return sched, percore, wx


def _build_program(sched, nchunks=512):
    tstart = sched["tstart"]
    ttot = sched["ttot"]

    EXP = mybir.ActivationFunctionType.Exp
    LN = mybir.ActivationFunctionType.Ln
    CP = mybir.ActivationFunctionType.Copy

    nc = bacc.Bacc("TRN2", target_bir_lowering=False, debug=False,
                   enable_asserts=False, num_devices=NCORES)

    d_attrT = nc.dram_tensor("attrT", [EC, ttot * P], BF16, kind="ExternalInput")
    d_ohall = nc.dram_tensor("ohall", [P, ttot * P], BF16, kind="ExternalInput")
    d_gidx = nc.dram_tensor("gidx", [P, ttot * 8], I16, kind="ExternalInput")
    d_par0 = nc.dram_tensor("par0", [P, ttot], F32, kind="ExternalInput")
    d_par1 = nc.dram_tensor("par1", [P, ttot], F32, kind="ExternalInput")
    d_featsT = nc.dram_tensor("featsT", [IN, NPAD], F32, kind="ExternalInput")
    d_ptembT = nc.dram_tensor("ptembT", [H, NPAD], F32, kind="ExternalInput")
    d_nW1T = nc.dram_tensor("nW1T", [L, EC, FLT], BF16, kind="ExternalInput")
    d_nb1 = nc.dram_tensor("nb1", [FLT, L], F32, kind="ExternalInput")
    d_nW2T = nc.dram_tensor("nW2T", [L, FLT, FLT], BF16, kind="ExternalInput")
    d_nb2row = nc.dram_tensor("nb2row", [L, P, FLT], BF16, kind="ExternalInput")
    d_l1WT = nc.dram_tensor("l1WT", [L, H, FLT], BF16, kind="ExternalInput")
    d_l2WT = nc.dram_tensor("l2WT", [L, FLT, H], BF16, kind="ExternalInput")
    d_l2b = nc.dram_tensor("l2b", [H, L], F32, kind="ExternalInput")
    d_lWT = nc.dram_tensor("lWT", [L, H, H], BF16, kind="ExternalInput")
    d_lbp = nc.dram_tensor("lbp", [H, L], F32, kind="ExternalInput")
    d_emblinT = nc.dram_tensor("emblinT", [IN, H], F32, kind="ExternalInput")

    d_hout = nc.dram_tensor("hout", [NPAD, H], F32, kind="ExternalOutput")

    d_xlocal = nc.dram_tensor("xlocal", [NPAD, H], BF16, kind="Internal")
    d_xtable = nc.dram_tensor("xtable", [NCORES * NPAD, H], BF16,
                              kind="Internal", addr_space="Shared")
    # same bytes viewed as node pairs: row k = [x[2k] | x[2k+1]]
    xpair_v = d_xtable[:].rearrange("(k two) f -> k (two f)", two=2)

    with tile.TileContext(nc) as tc:
        with (
            tc.tile_pool(name="const", bufs=1) as cpool,
            tc.tile_pool(name="attr", bufs=2) as p_attr,
            tc.tile_pool(name="ohp", bufs=2) as p_oh,
            tc.tile_pool(name="xg", bufs=2) as p_xg,
            tc.tile_pool(name="ssp1", bufs=2) as p_ssp,
            tc.tile_pool(name="small", bufs=4) as p_small,
            tc.tile_pool(name="mx", bufs=4) as p_mx,
            tc.tile_pool(name="flush", bufs=2) as p_flush,
            tc.tile_pool(name="exp", bufs=2) as p_exp,
            tc.tile_pool(name="pt1", bufs=2, space="PSUM") as p_t1,
            tc.tile_pool(name="pw", bufs=2, space="PSUM") as p_W,
            tc.tile_pool(name="pagg", bufs=2, space="PSUM") as p_agg,
            tc.tile_pool(name="pmisc", bufs=2, space="PSUM") as p_misc,
        ):
            # ---- constants in SBUF ----
            def cload(dram_ap, shape, dt, tag):
                t = cpool.tile(shape, dt, tag=tag)
                nc.sync.dma_start(out=t[:], in_=dram_ap)
                return t

            c_gidx = cload(d_gidx[:], [P, ttot * 8], I16, "gidx")
            c_par0 = cload(d_par0[:], [P, ttot], F32, "par0")
            c_par1 = cload(d_par1[:], [P, ttot], F32, "par1")
            c_nb1 = cload(d_nb1[:], [FLT, L], F32, "nb1")
            c_l2b = cload(d_l2b[:], [H, L], F32, "l2b")
            c_lbp = cload(d_lbp[:], [H, L], F32, "lbp")
            c_emblinT = cload(d_emblinT[:], [IN, H], F32, "emblinT")
            c_featsT = cload(d_featsT[:], [IN, NPAD], F32, "featsT")
            c_ptembT = cload(d_ptembT[:], [H, NPAD], F32, "ptembT")
            c_nW1T = [cload(d_nW1T[l], [EC, FLT], BF16, f"nW1T{l}") for l in range(L)]
            c_nW2T = [cload(d_nW2T[l], [FLT, FLT], BF16, f"nW2T{l}") for l in range(L)]
            c_nb2row = [cload(d_nb2row[l], [P, FLT], BF16, f"nb2row{l}") for l in range(L)]
            c_l1WT = [cload(d_l1WT[l], [H, FLT], BF16, f"l1WT{l}") for l in range(L)]
            c_l2WT = [cload(d_l2WT[l], [FLT, H], BF16, f"l2WT{l}") for l in range(L)]
            c_lWT = [cload(d_lWT[l], [H, H], BF16, f"lWT{l}") for l in range(L)]

            c_ident = cpool.tile([P, P], F32, tag="ident")
            make_identity(nc, c_ident[:])

            hT = cpool.tile([P, NPAD], F32, tag="hT")
            hbf = cpool.tile([P, NPAD], BF16, tag="hbf")
            x_sb = cpool.tile([P, NW, P], BF16, tag="x_sb")
            agg_sb = cpool.tile([P, NW, P], F32, tag="agg_sb")
            aggT_sb = cpool.tile([P, NPAD], BF16, tag="aggT_sb")
            s_sb = cpool.tile([P, NPAD], BF16, tag="s_sb")

            nck = NPAD // nchunks   # node-dim chunks
            tpc = nchunks // P      # x tiles per chunk

            xlocal_v = d_xlocal[:].rearrange("(t p) f -> p t f", p=P)
            hout_v = d_hout[:].rearrange("(t p) f -> p t f", p=P)

            def emit_x_chunk(l, k):
                """x = h @ lin1.T for node chunk k, DMA'd to xlocal."""
                sl = bass.ts(k, nchunks)
                nc.vector.tensor_copy(out=hbf[:, sl], in_=hT[:, sl])
                for t in range(k * tpc, (k + 1) * tpc):
                    ps = p_misc.tile([P, FLT], F32, tag="misc")
                    nc.tensor.matmul(out=ps[:], lhsT=hbf[:, bass.ts(t, P)],
                                     rhs=c_l1WT[l][:], start=True, stop=True)
                    nc.scalar.activation(x_sb[:, t, :], ps[:], CP)
                nc.sync.dma_start(
                    out=xlocal_v[:, k * tpc : (k + 1) * tpc, :],
                    in_=x_sb[:, k * tpc : (k + 1) * tpc, :])

            def emit_collective():
                nc.gpsimd.collective_compute(
                    "AllGather", mybir.AluOpType.bypass,
                    replica_groups=[list(range(NCORES))],
                    ins=[d_xlocal[:]], outs=[d_xtable[:]],
                )

            # ---- h0 = feats @ emblin.T + (ptemb + emblin_b), then x0 ----
            for k in range(nck):
                sl = bass.ts(k, nchunks)
                ps = p_t1.tile([P, nchunks], F32, tag="t1")
                nc.tensor.matmul(out=ps[:], lhsT=c_emblinT[:], rhs=c_featsT[:, sl],
                                 start=True, stop=True)
                nc.vector.tensor_tensor(out=hT[:, sl], in0=ps[:],
                                        in1=c_ptembT[:, sl],
                                        op=mybir.AluOpType.add)
                emit_x_chunk(0, k)
            emit_collective()

            for l in range(L):
                # ---- edge phase, one destination window at a time ----
                for w in range(NW):
                    t0, t1 = int(tstart[w]), int(tstart[w + 1])
                    tw = t1 - t0
                    ne = tw * P
                    attr_t = p_attr.tile([EC, ne], BF16, tag="attr")
                    nc.sync.dma_start(out=attr_t[:],
                                      in_=d_attrT[:, t0 * P : t1 * P])
                    oh_t = p_oh.tile([P, ne], BF16, tag="oh")
                    nc.sync.dma_start(out=oh_t[:],
                                      in_=d_ohall[:, t0 * P : t1 * P])
                    # batched pair-gather, <=1024 indices per call (512B rows
                    # use 2 descriptors; the SWDGE ring holds 128/engine)
                    xg_t = p_xg.tile([P, tw, 2 * P], BF16, tag="xg")
                    for g0 in range(0, tw, GMAX):
                        g1 = min(tw, g0 + GMAX)
                        nc.gpsimd.dma_gather(
                            out_ap=xg_t[:, g0:g1, :],
                            in_ap=xpair_v,
                            idxs_ap=c_gidx[:, (t0 + g0) * 8 : (t0 + g1) * 8],
                            num_idxs=(g1 - g0) * P, num_idxs_reg=(g1 - g0) * P,
                            elem_size=2 * P, single_packet=False)
                    ssp1_t = p_ssp.tile([P, ne], BF16, tag="ssp1")
                    for j in range(0, ne, 512):
                        je = min(ne, j + 512)
                        pt1 = p_t1.tile([P, je - j], F32, tag="t1")
                        nc.tensor.matmul(out=pt1[:], lhsT=c_nW1T[l][:],
                                         rhs=attr_t[:, j:je],
                                         start=True, stop=True)
                        # softplus(x + b) = Ln(Exp(x + b) + 1)
                        ex = p_exp.tile([P, je - j], F32, tag="exp")
                        nc.scalar.activation(ex[:], pt1[:], EXP,
                                             bias=c_nb1[:, l : l + 1])
                        nc.scalar.activation(ssp1_t[:, j:je], ex[:], LN,
                                             bias=1.0)

                    pAB = p_agg.tile([P, 2 * P], F32, tag="agg")
                    for t in range(tw):
                        pW = p_W.tile([P, P], F32, tag="w")
                        nc.tensor.matmul(out=pW[:], lhsT=ssp1_t[:, bass.ts(t, P)],
                                         rhs=c_nW2T[l][:], start=True, stop=True)
                        # parity select into mx right half:
                        #   xg = par0*pairL + par1*pairR
                        t1s = p_small.tile([P, P], BF16, tag="t1s")
                        nc.scalar.activation(t1s[:], xg_t[:, t, 0:P], CP,
                                             scale=c_par0[:, t0 + t : t0 + t + 1])
                        mx = p_mx.tile([P, 2 * P], BF16, tag="mx")
                        nc.vector.scalar_tensor_tensor(
                            out=mx[:, P : 2 * P], in0=xg_t[:, t, P : 2 * P],
                            scalar=c_par1[:, t0 + t : t0 + t + 1], in1=t1s[:],
                            op0=mybir.AluOpType.mult, op1=mybir.AluOpType.add)
                        nc.vector.tensor_tensor(out=mx[:, 0:P], in0=pW[:],
                                                in1=mx[:, P : 2 * P],
                                                op=mybir.AluOpType.mult)
                        nc.tensor.matmul(out=pAB[:], lhsT=oh_t[:, bass.ts(t, P)],
                                         rhs=mx[:], start=(t == 0),
                                         stop=(t == tw - 1))
                    tmp = p_flush.tile([P, P], F32, tag="tmp")
                    nc.vector.tensor_tensor(out=tmp[:], in0=pAB[:, P : 2 * P],
                                            in1=c_nb2row[l][:],
                                            op=mybir.AluOpType.mult)
                    nc.vector.tensor_tensor(out=agg_sb[:, w, :],
                                            in0=pAB[:, 0:P],
                                            in1=tmp[:], op=mybir.AluOpType.add)

                    # ---- agg [n,f] -> aggT [f,n] for this window ----
                    pt = p_misc.tile([P, P], F32, tag="misc")
                    nc.tensor.transpose(out=pt[:], in_=agg_sb[:, w, :],
                                        identity=c_ident[:])
                    nc.scalar.activation(aggT_sb[:, bass.ts(w, P)], pt[:], CP)

                    # ---- node path chunk once its 4 windows are done ----
                    if w % tpc == tpc - 1:
                        k = w // tpc
                        sl = bass.ts(k, nchunks)
                        p2 = p_t1.tile([P, nchunks], F32, tag="t1")
                        nc.tensor.matmul(out=p2[:], lhsT=c_l2WT[l][:],
                                         rhs=aggT_sb[:, sl],
                                         start=True, stop=True)
                        ex2 = p_exp.tile([P, nchunks], F32, tag="exp")
                        nc.scalar.activation(ex2[:], p2[:], EXP,
                                             bias=c_l2b[:, l : l + 1])
                        nc.scalar.activation(s_sb[:, sl], ex2[:], LN,
                                             bias=1.0)
                        p3 = p_W.tile([P, nchunks], F32, tag="w")
                        nc.tensor.matmul(out=p3[:], lhsT=c_lWT[l][:],
                                         rhs=s_sb[:, sl], start=True, stop=True)
                        tr = p_flush.tile([P, nchunks], F32, tag="tr")
                        nc.vector.tensor_scalar(out=tr[:], in0=p3[:],
                                                scalar1=c_lbp[:, l : l + 1],
                                                scalar2=None,
                                                op0=mybir.AluOpType.add)
                        nc.vector.tensor_tensor(out=hT[:, sl], in0=hT[:, sl],
                                                in1=tr[:],
                                                op=mybir.AluOpType.add)
                        if l + 1 < L:
                            emit_x_chunk(l + 1, k)
                if l + 1 < L:
                    emit_collective()

            # ---- output: transpose hT back to [node, feat] ----
            for t in range(NW):
                pt = p_misc.tile([P, P], F32, tag="misc")
                nc.tensor.transpose(out=pt[:], in_=hT[:, bass.ts(t, P)],
                                    identity=c_ident[:])
                nc.vector.tensor_copy(out=agg_sb[:, t, :], in_=pt[:])
            nc.sync.dma_start(out=hout_v, in_=agg_sb[:])

    nc.compile()
    return nc


def kernel(**inputs):
    sched, percore, wx = _host_prep(inputs)
    nc = _build_program(sched)

    in_maps = []
    for c in range(NCORES):
        m = {k: np.ascontiguousarray(percore[k][c]) for k in percore}
        for k, v in wx.items():
            m[k] = v
        in_maps.append(m)

    res = run_bass_kernel_spmd(nc, in_maps, core_ids=list(range(NCORES)))
    out = np.empty((N, H), np.float32)
    for c in range(NCORES):
        out[c * NLOC : (c + 1) * NLOC] = res.results[c]["hout"][:NLOC]
    return out

